# revision 32
# baseline (speedup 1.0000x reference)
"""Trainium2 Bass kernel for nn_AttnBlock (GroupNorm + single-head attention
block over [b=4, c=512, l=4096] fp32, 8 NeuronCores).

Sharding: core = (batch, query-half). Each core gets one batch item with its
query half permuted to columns 0..2047 (GroupNorm/attention are invariant to
a consistent permutation of l), computes the full block for its 2048 query
positions, and the host reassembles the [4, 512, 4096] output.

Design (vs the v1 baseline, 346 -> ~222 us):
  - Weights pre-scaled by WS=16 and cast to fp8e4m3 on the HOST (kills 3 MB
    of prologue DMA + all on-chip weight casts); x shipped as fp8 in the
    DoubleRow layout and used RAW as the matmul operand - no separate
    GN-cast pass at all. All power-of-two compensations fold into existing
    scalar constants (exp scale, o_sb shrink, bias scales).
  - GroupNorm folded into the weights: per-input-channel scale m multiplies
    wk/wq/wv fp8 rows after stats; the GN bias term rides tiny N=1 PE
    matmuls (K's bias provably cancels in the softmax; V's folds into bp3;
    Q's lands in the PSUM-evacuation bias add).
  - GroupNorm stats split across engines and pipelined per channel block:
    DVE bn_stats for blocks 0/1/3a + ACT Square/Identity with accum_out for
    blocks 2/3b, x blocks arriving via three parallel DMA rings.
  - Attention software-pipelined: next step's S^T matmuls are emitted before
    this step's s/O matmuls so the PE never waits on exp() - the attention
    phase measures ~100% PE occupancy (<1 us total gaps).
  - s row-sums via an M=128 all-ones stationary => s broadcast to all
    partitions for free; 1/s on DVE directly; no PE broadcast matmul, no
    rank-1 bias-inject matmuls (bp3 pre-added into the residual tile).
  - Residual x streamed in f32 during the attention phase (HBM idle then);
    out = proj*rinv + (res+bp3) is a 2-op DVE chain.
  - PSUM: 3 (S^T) + 1 (s/bias) + 4 (O/proj) banks; PE warmup dummies paced
    by the stats stream pre-trigger the HAM clock un-throttle.
"""
import os
import sys
from contextlib import ExitStack

import numpy as np

sys.path.insert(0, "/opt/trn_rl_repo")

import concourse.bass as bass
import concourse.tile as tile
from concourse import bacc, mybir

F32 = mybir.dt.float32
BF16 = mybir.dt.bfloat16
F8 = mybir.dt.float8e4

B, C, L = 4, 512, 4096
NQ = L // 2          # queries per core
P = 128
CO = C // P          # 4 channel blocks
NJT = L // P         # 32 j-tiles
NIB = NQ // 512      # 4 i-blocks
NLC = L // 512       # 8 l-chunks
NG = 32              # groups
GSZ = C // NG        # 16 channels per group
GPP = P // GSZ       # 8 groups per 128 partitions
EPS = 1e-6
SCALE = float(C) ** -0.5
WS = 16.0            # host-side weight scale (power of two)
ESCALE = SCALE / (WS * WS)   # exp() input scale
OSC = 1.0 / 256.0    # O_u -> fp8 shrink, with the 1/WS^2 proj
                     # compensation folded in: out = pj * (1/s) exactly
HC = L // 2          # half-columns per x chunk

AF = mybir.ActivationFunctionType
ALU = mybir.AluOpType
DR = mybir.MatmulPerfMode.DoubleRow


def build_program():
    nc = bacc.Bacc("TRN2")
    x_d = nc.declare_dram_parameter("xb", [C, L], F8, isOutput=False)
    xr_d = nc.declare_dram_parameter("xr", [C, NQ], F32, isOutput=False)
    wq_d = nc.declare_dram_parameter("wq8", [C, C], F8, isOutput=False)
    wk_d = nc.declare_dram_parameter("wk8", [C, C], F8, isOutput=False)
    wv_d = nc.declare_dram_parameter("wv8", [C, C], F8, isOutput=False)
    wp_d = nc.declare_dram_parameter("wp8", [C, C], F8, isOutput=False)
    gns_d = nc.declare_dram_parameter("gn_scale", [C], F32, isOutput=False)
    gnb_d = nc.declare_dram_parameter("gn_bias", [C], F32, isOutput=False)
    bqw_d = nc.declare_dram_parameter("bqw", [C], F32, isOutput=False)
    bvw_d = nc.declare_dram_parameter("bvw", [C], F32, isOutput=False)
    bp_d = nc.declare_dram_parameter("bp", [C], F32, isOutput=False)
    gm_d = nc.declare_dram_parameter("gmat", [P, GPP], F32, isOutput=False)
    gt_d = nc.declare_dram_parameter("gtmat", [GPP, P], F32, isOutput=False)
    out_d = nc.declare_dram_parameter("out", [C, NQ], F32, isOutput=True)

    with tile.TileContext(nc) as tc:
        attn_block(tc, x_d, xr_d, wq_d, wk_d, wv_d, wp_d, gns_d, gnb_d,
                   bqw_d, bvw_d, bp_d, gm_d, gt_d, out_d)
    nc.compile()
    return nc


def attn_block(tc, x_d, xr_d, wq_d, wk_d, wv_d, wp_d, gns_d, gnb_d,
               bqw_d, bvw_d, bp_d, gm_d, gt_d, out_d):
    nc = tc.nc
    x_v = x_d.ap().rearrange("(a b p) l -> p a b l", a=2, b=2, p=P)
    xr_v = xr_d.ap().rearrange("(o p) i -> p o i", p=P)
    out_v = out_d.ap().rearrange("(o p) i -> p o i", p=P)

    with ExitStack() as ctx:
        # ---- persistent pools (whole kernel) ----
        big = ctx.enter_context(tc.tile_pool(name="big", bufs=1))
        small = ctx.enter_context(tc.tile_pool(name="small", bufs=1))

        x8 = big.tile([P, 2, 2, L], F8, tag="x8")      # GN-normalized x, fp8
        k8 = big.tile([P, 2, 2, L], F8, tag="k8")
        q8 = big.tile([P, 2, 2, NQ], F8, tag="q8")
        vt8 = big.tile([P, NJT // 2, 2, C], F8, tag="vt8")
        wq8 = big.tile([P, 2, 2, C], F8, tag="wq8")
        wk8 = big.tile([P, 2, 2, C], F8, tag="wk8")
        wv8 = big.tile([P, 2, 2, C], F8, tag="wv8")
        wp8 = big.tile([P, 2, 2, C], F8, tag="wp8")

        gns = small.tile([P, CO], F32, tag="gns")
        gnb = small.tile([P, CO], F32, tag="gnb")
        bqw = small.tile([P, CO], F32, tag="bqw")
        bp_s = small.tile([P, CO], F32, tag="bps")
        bp3 = small.tile([P, CO], F32, tag="bp3")
        bvw = small.tile([P, CO], F32, tag="bvw")
        bv2_8 = small.tile([P, 2, 2], F8, tag="bv28")
        a8 = small.tile([P, CO], F8, tag="a8")
        bq2 = small.tile([P, CO], F32, tag="bq2")
        m44 = small.tile([P, CO], F32, tag="m44")
        a44 = small.tile([P, CO], F32, tag="a44")
        gm_t = small.tile([P, GPP], F32, tag="gmt")
        gt_t = small.tile([GPP, P], F32, tag="gtt")
        ones_p = small.tile([P, 2, P], F8, tag="onesp")
        nc.vector.memset(ones_p, 1.0)
        nshift = small.tile([P, 1], F32, tag="nshift")
        nc.vector.memset(nshift, -3.0)
        eps_t = small.tile([GPP, 1], F32, tag="eps")
        nc.vector.memset(eps_t, EPS)
        warm8 = small.tile([P, 512], F8, tag="warm8")
        nc.vector.memset(warm8, 1.0)

        # static PSUM layout: 3 + 1 + 4 = 8 banks, shared across phases
        ps_st = ctx.enter_context(
            tc.tile_pool(name="psst", bufs=3, space="PSUM"))
        ps_s = ctx.enter_context(
            tc.tile_pool(name="pss", bufs=1, space="PSUM"))
        ps_o = ctx.enter_context(
            tc.tile_pool(name="pso", bufs=4, space="PSUM"))

        # small/weight DMAs on the gpsimd ring (x uses sync+scalar rings)
        for v_d, v_t in ((gns_d, gns), (gnb_d, gnb), (bqw_d, bqw),
                         (bp_d, bp_s), (bvw_d, bvw)):
            nc.gpsimd.dma_start(out=v_t[:], in_=v_d.ap().rearrange(
                "(o p) -> p o", p=P))

        nc.gpsimd.dma_start(out=gm_t[:], in_=gm_d.ap())
        nc.gpsimd.dma_start(out=gt_t[:], in_=gt_d.ap())

        # ====== prologue: raw-fp8 x, stats on DVE+ACT, GN folded into W ======
        with ExitStack() as pctx:
            pro = pctx.enter_context(tc.tile_pool(name="pro", bufs=1))
            tiny_ps = ps_st
            qkv_ps = ps_o

            bnst = pro.tile([P, CO, 8, 6], F32, tag="bnst")
            mv = pro.tile([P, CO, 2], F32, tag="mv")
            st2 = pro.tile([P, CO, 2], F32, tag="st2")
            sc2 = pro.tile([P, CO, 1], F32, tag="sc2")
            grp = pro.tile([GPP, CO, 6], F32, tag="grp")
            asum2 = pro.tile([P, 2, 2], F32, tag="asum2")
            asum3 = pro.tile([P, 2, 1], F32, tag="asum3")
            red2 = pro.tile([P, 2], F32, tag="red2")
            scr2 = pro.tile([P, 2, HC], F32, tag="scr2")

            # x blocks land via 4 parallel DMA rings in DoubleRow layout;
            # raw fp8 x is used directly as the matmul operand (GroupNorm
            # scale rides the weights, bias via tiny matmuls).
            nc.sync.dma_start(out=x8[:, 0, 0, :], in_=x_v[:, 0, 0, :])
            nc.scalar.dma_start(out=x8[:, 1, 0, :], in_=x_v[:, 1, 0, :])
            nc.gpsimd.dma_start(out=x8[:, 0, 1, :], in_=x_v[:, 0, 1, :])
            nc.scalar.dma_start(out=x8[:, 1, 1, :], in_=x_v[:, 1, 1, :])

            for w_d, w_t in ((wk_d, wk8), (wq_d, wq8), (wv_d, wv8),
                             (wp_d, wp8)):
                nc.gpsimd.dma_start(out=w_t[:], in_=w_d.ap().rearrange(
                    "(a b p) c -> p a b c", a=2, b=2, p=P))

            # block 2 stats on ACT, emitted first so they lead its queue:
            # two 2048-wide passes amortize the accumulator-read overhead
            for half in range(2):
                cols = slice(half * HC, (half + 1) * HC)
                nc.scalar.activation(out=scr2[:, half, :],
                                     in_=x8[:, 1, 0, cols], func=AF.Square,
                                     accum_out=asum2[:, 1, half:half + 1])
                nc.scalar.activation(out=scr2[:, half, :],
                                     in_=x8[:, 1, 0, cols], func=AF.Identity,
                                     accum_out=asum2[:, 0, half:half + 1])

            def block_chunk(o, h):
                return x8[:, o // 2, o % 2, h * 512:(h + 1) * 512]

            def combine(o):
                """group mean/rstd -> m44/a44/a8 col o (no folds here:
                keeping ACT's queue to bare sqrt ops lets the four chains
                pipeline instead of serializing behind weight folds)."""
                g_ps = tiny_ps.tile([GPP, 2], F32, tag="mm")
                nc.tensor.matmul(g_ps, lhsT=gm_t, rhs=st2[:, o, :],
                                 start=True, stop=True)
                nc.vector.tensor_copy(grp[:, o, 0:1], g_ps[:, 0:1])
                nc.vector.tensor_mul(grp[:, o, 2:3], grp[:, o, 0:1],
                                     grp[:, o, 0:1])
                nc.vector.tensor_sub(grp[:, o, 2:3], g_ps[:, 1:2],
                                     grp[:, o, 2:3])
                nc.scalar.activation(out=grp[:, o, 3:4], in_=grp[:, o, 2:3],
                                     func=AF.Sqrt, bias=eps_t)
                nc.vector.reciprocal_approx_accurate(
                    grp[:, o, 1:2], grp[:, o, 3:4], grp[:, o, 4:5])
                bc_ps = tiny_ps.tile([P, 2], F32, tag="mm")
                nc.tensor.matmul(bc_ps, lhsT=gt_t, rhs=grp[:, o, 0:2],
                                 start=True, stop=True)
                mcol = m44[:, o:o + 1]
                acol = a44[:, o:o + 1]
                nc.vector.tensor_mul(mcol, bc_ps[:, 1:2], gns[:, o:o + 1])
                nc.vector.tensor_mul(acol, bc_ps[:, 0:1], mcol)
                nc.vector.tensor_sub(acol, gnb[:, o:o + 1], acol)
                nc.vector.tensor_scalar_mul(a8[:, o:o + 1], acol, 64.0)

            def fold_and_bias(o):
                mcol = m44[:, o:o + 1]
                nc.scalar.activation(out=wk8[:, o // 2, o % 2, :],
                                     in_=wk8[:, o // 2, o % 2, :],
                                     func=AF.Copy, scale=mcol)
                for oc in range(CO):
                    nc.tensor.matmul(
                        bias_ps[:, oc:oc + 1],
                        lhsT=wq8[:, o // 2, o % 2, oc * P:(oc + 1) * P],
                        rhs=a8[:, o:o + 1],
                        start=(o == 0), stop=(o == CO - 1))
                for oc in range(CO):
                    nc.tensor.matmul(
                        bias_ps[:, 4 + oc:5 + oc],
                        lhsT=wv8[:, o // 2, o % 2, oc * P:(oc + 1) * P],
                        rhs=a8[:, o:o + 1],
                        start=(o == 0), stop=(o == CO - 1))

            bias_ps = ps_s.tile([P, 8], F32, tag="srow")
            ci = 0
            for o in (0, 1):  # DVE-owned blocks
                for hh in range(2):
                    for h in range(4):
                        nc.vector.bn_stats(
                            out=bnst[:, o, hh * 4 + h, :],
                            in_=block_chunk(o, hh * 4 + h))
                    # HAM warmup: dummy matmuls paced by the stats stream
                    nc.vector.tensor_copy(warm8[:, ci * 4:ci * 4 + 4],
                                          bnst[:, o, hh * 4 + 3, 0:4])
                    for _ in range(2 + 2 * o):
                        wm_ps = tiny_ps.tile([P, 512], F32, tag="mm")
                        nc.tensor.matmul(wm_ps, lhsT=warm8[:, 0:P],
                                         rhs=warm8[:], start=True, stop=True)
                    ci += 1
                nc.vector.bn_aggr(out=mv[:, o, :], in_=bnst[:, o, :, :])
                nc.vector.tensor_copy(st2[:, o, 0:1], mv[:, o, 0:1])
                nc.vector.tensor_mul(sc2[:, o, :], mv[:, o, 0:1],
                                     mv[:, o, 0:1])
                nc.vector.tensor_add(st2[:, o, 1:2], sc2[:, o, :],
                                     mv[:, o, 1:2])

            # block 2 merge (ACT accumulators -> mean / E[x^2])
            nc.vector.tensor_reduce(out=red2, in_=asum2,
                                    axis=mybir.AxisListType.X,
                                    op=ALU.add)
            nc.vector.tensor_scalar_mul(st2[:, 2, :], red2, 1.0 / L)

            # block 3 fully on DVE
            for h in range(8):
                nc.vector.bn_stats(out=bnst[:, 3, h, :],
                                   in_=block_chunk(3, h))
                if h in (1, 4):
                    nc.vector.tensor_copy(warm8[:, 32 + h * 4:36 + h * 4],
                                          bnst[:, 3, h, 0:4])
                    for _ in range(3):
                        wm_ps = tiny_ps.tile([P, 512], F32, tag="mm")
                        nc.tensor.matmul(wm_ps, lhsT=warm8[:, 0:P],
                                         rhs=warm8[:], start=True,
                                         stop=True)
            nc.vector.bn_aggr(out=mv[:, 3, :], in_=bnst[:, 3, :, :])
            nc.vector.tensor_copy(st2[:, 3, 0:1], mv[:, 3, 0:1])
            nc.vector.tensor_mul(sc2[:, 3, :], mv[:, 3, 0:1], mv[:, 3, 0:1])
            nc.vector.tensor_add(st2[:, 3, 1:2], sc2[:, 3, :],
                                 mv[:, 3, 1:2])

            # all stats emitted: now the four combine chains pipeline with
            # nothing else in the ACT/DVE queues, then folds + bias matmuls
            for o in range(CO):
                combine(o)
            for o in range(CO):
                fold_and_bias(o)

            # ---- finish bias path: bq2 / bv2_8 from accumulated matmuls ----
            for oc in range(CO):
                nc.vector.tensor_scalar(out=bq2[:, oc:oc + 1],
                                        in0=bias_ps[:, oc:oc + 1],
                                        scalar1=1.0 / 64.0,
                                        scalar2=bqw[:, oc:oc + 1],
                                        op0=ALU.mult, op1=ALU.add)
                nc.vector.tensor_scalar(
                    out=bv2_8[:, oc // 2, oc % 2:oc % 2 + 1],
                    in0=bias_ps[:, 4 + oc:5 + oc],
                    scalar1=1.0 / 64.0, scalar2=bvw[:, oc:oc + 1],
                    op0=ALU.mult, op1=ALU.add)
            # fold GN scale into wq/wv now that the bias matmuls read them
            for i, o in enumerate(range(CO)):
                eng = nc.vector if i % 2 == 0 else None
                mcol = m44[:, o:o + 1]
                if eng is None:
                    nc.scalar.activation(out=wq8[:, o // 2, o % 2, :],
                                         in_=wq8[:, o // 2, o % 2, :],
                                         func=AF.Copy, scale=mcol)
                    nc.scalar.activation(out=wv8[:, o // 2, o % 2, :],
                                         in_=wv8[:, o // 2, o % 2, :],
                                         func=AF.Copy, scale=mcol)
                else:
                    eng.tensor_scalar_mul(wq8[:, o // 2, o % 2, :],
                                          wq8[:, o // 2, o % 2, :], mcol)
                    eng.tensor_scalar_mul(wv8[:, o // 2, o % 2, :],
                                          wv8[:, o // 2, o % 2, :], mcol)

            # ---- bp3 = bp + wp @ (wv a + bv) : post-normalize bias ----
            for oc in range(CO):
                b_ps = tiny_ps.tile([P, 1], F32, tag="mm")
                for o in range(CO):
                    nc.tensor.matmul(b_ps,
                                     lhsT=wp8[:, o // 2, o % 2,
                                              oc * P:(oc + 1) * P],
                                     rhs=bv2_8[:, o // 2, o % 2:o % 2 + 1],
                                     start=(o == 0), stop=(o == CO - 1))
                nc.vector.tensor_scalar(out=bp3[:, oc:oc + 1], in0=b_ps,
                                        scalar1=1.0 / (WS * WS),
                                        scalar2=bp_s[:, oc:oc + 1],
                                        op0=ALU.mult, op1=ALU.add)

            # ---- Q / K / V^T from resident raw-fp8 x ----
            ev = 0
            for lc in range(NLC):
                l0 = lc * 512
                for oc in range(CO):
                    kp = qkv_ps.tile([P, 512], F32, tag="acc")
                    for pr in range(2):
                        nc.tensor.matmul(
                            kp, lhsT=wk8[:, pr, :, oc * P:(oc + 1) * P],
                            rhs=x8[:, pr, :, l0:l0 + 512],
                            start=(pr == 0), stop=(pr == 1), perf_mode=DR)
                    dst = k8[:, oc // 2, oc % 2, l0:l0 + 512]
                    if ev % 2 == 0:
                        nc.scalar.activation(out=dst, in_=kp, func=AF.Copy)
                    else:
                        nc.vector.tensor_copy(dst, kp)
                    ev += 1
                for jt in range(4):
                    j0 = l0 + jt * P
                    jtg = lc * 4 + jt
                    vp = qkv_ps.tile([P, C], F32, tag="acc")
                    for pr in range(2):
                        nc.tensor.matmul(
                            vp, lhsT=x8[:, pr, :, j0:j0 + P],
                            rhs=wv8[:, pr, :, :],
                            start=(pr == 0), stop=(pr == 1), perf_mode=DR)
                    dst = vt8[:, jtg // 2, jtg % 2, :]
                    if ev % 2 == 0:
                        nc.scalar.activation(out=dst, in_=vp, func=AF.Copy)
                    else:
                        nc.vector.tensor_copy(dst, vp)
                    ev += 1
                if lc < NIB:
                    for oc in range(CO):
                        qp = qkv_ps.tile([P, 512], F32, tag="acc")
                        for pr in range(2):
                            nc.tensor.matmul(
                                qp,
                                lhsT=wq8[:, pr, :, oc * P:(oc + 1) * P],
                                rhs=x8[:, pr, :, l0:l0 + 512],
                                start=(pr == 0), stop=(pr == 1),
                                perf_mode=DR)
                        dst = q8[:, oc // 2, oc % 2, l0:l0 + 512]
                        if ev % 2 == 0:
                            nc.scalar.activation(out=dst, in_=qp,
                                                 func=AF.Identity,
                                                 bias=bq2[:, oc:oc + 1])
                        else:
                            nc.vector.tensor_scalar_add(dst, qp,
                                                        bq2[:, oc:oc + 1])
                        ev += 1

        # ================= attention + proj per i-block =================
        with ExitStack() as actx:
            p_pool = actx.enter_context(tc.tile_pool(name="ppool", bufs=4))
            res_pool = actx.enter_context(tc.tile_pool(name="resp", bufs=2))
            osb_pool = actx.enter_context(tc.tile_pool(name="osb", bufs=2))
            out_pool = actx.enter_context(tc.tile_pool(name="outp", bufs=4))
            rinv_pool = actx.enter_context(tc.tile_pool(name="rinvp", bufs=2))

            NT = NJT // 2
            steps = [(ib, t) for ib in range(NIB) for t in range(NT)]

            def emit_scores(ib, t):
                """S^T matmuls + exp for step (ib, t) -> p_f8 tile."""
                i0 = ib * 512
                p_f8 = p_pool.tile([P, 2, 512], F8, tag="pbf")
                for ko in range(2):
                    jt = 2 * t + ko
                    st_ps = ps_st.tile([P, 512], F32, tag="mm")
                    for pr in range(2):
                        nc.tensor.matmul(
                            st_ps,
                            lhsT=k8[:, pr, :, jt * P:(jt + 1) * P],
                            rhs=q8[:, pr, :, i0:i0 + 512],
                            start=(pr == 0), stop=(pr == 1), perf_mode=DR)
                    # exp(S/sqrt(c) - 3): shift keeps P in fp8e4 range,
                    # cancels between the s-normalization and bp3 path.
                    nc.scalar.activation(
                        out=p_f8[:, ko, :], in_=st_ps, func=AF.Exp,
                        bias=nshift, scale=ESCALE)
                return p_f8

            s_ps = None
            o_ps = None
            p_cur = emit_scores(0, 0)
            for idx, (ib, t) in enumerate(steps):
                i0 = ib * 512
                if t == 0:
                    s_ps = ps_s.tile([P, 512], F32, tag="srow")
                    o_ps = [ps_o.tile([P, 512], F32, tag="acc",
                                      name=f"oacc{cc}") for cc in range(CO)]
                    res = res_pool.tile([P, CO, 512], F32, tag="res")
                    nc.sync.dma_start(out=res[:],
                                      in_=xr_v[:, :, i0:i0 + 512])
                    for cc in range(CO):
                        nc.vector.tensor_scalar_add(res[:, cc, :],
                                                    res[:, cc, :],
                                                    bp3[:, cc:cc + 1])
                # prefetch next step's scores: keeps PE fed while this
                # step's exp() drains on ACT
                p_next = (emit_scores(*steps[idx + 1])
                          if idx + 1 < len(steps) else None)
                nc.tensor.matmul(s_ps, lhsT=ones_p, rhs=p_cur,
                                 start=(t == 0), stop=(t == NT - 1),
                                 perf_mode=DR)
                for cc in range(CO):
                    nc.tensor.matmul(
                        o_ps[cc], lhsT=vt8[:, t, :, cc * P:(cc + 1) * P],
                        rhs=p_cur, start=(t == 0), stop=(t == NT - 1),
                        perf_mode=DR)
                p_cur = p_next
                if t < NT - 1:
                    continue
                # ---- epilogue: 1/s, fp8 O, proj, residual ----
                rinv = rinv_pool.tile([P, 512], F32, tag="rinv")
                nc.vector.reciprocal_approx_fast(rinv, s_ps)
                o_sb = osb_pool.tile([P, 2, 2, 512], F8, tag="osb")
                for cc in range(CO):
                    dst = o_sb[:, cc // 2, cc % 2, :]
                    if cc % 2 == 0:
                        nc.scalar.activation(out=dst, in_=o_ps[cc],
                                             func=AF.Copy, scale=OSC)
                    else:
                        nc.vector.tensor_scalar_mul(dst, o_ps[cc], OSC)
                for oc in range(CO):
                    pj_ps = ps_o.tile([P, 512], F32, tag="acc",
                                      name=f"pj{oc}")
                    for pr in range(2):
                        nc.tensor.matmul(
                            pj_ps,
                            lhsT=wp8[:, pr, :, oc * P:(oc + 1) * P],
                            rhs=o_sb[:, pr, :, :],
                            start=(pr == 0), stop=(pr == 1), perf_mode=DR)
                    out_t = out_pool.tile([P, 512], F32, tag="outt")
                    nc.vector.tensor_mul(out_t, pj_ps, rinv)
                    nc.vector.tensor_add(out_t, out_t, res[:, oc, :])
                    nc.sync.dma_start(out=out_v[:, oc, i0:i0 + 512],
                                      in_=out_t)


def kernel(**inputs):
    import ml_dtypes

    F8NP = ml_dtypes.float8_e4m3fn
    BF16NP = ml_dtypes.bfloat16
    x = np.ascontiguousarray(np.asarray(inputs["x"], np.float32))
    args = {}
    for nm, w in (("wq8", inputs["wq"]), ("wk8", inputs["wk"]),
                  ("wv8", inputs["wv"]), ("wp8", inputs["wp"])):
        wT = np.asarray(w, np.float32).T * WS
        args[nm] = np.ascontiguousarray(wT.astype(F8NP))
    args["gn_scale"] = np.asarray(inputs["gn_scale"], np.float32)
    args["gn_bias"] = np.asarray(inputs["gn_bias"], np.float32)
    args["bqw"] = np.asarray(inputs["bq"], np.float32) * np.float32(WS)
    args["bvw"] = np.asarray(inputs["bv"], np.float32) * np.float32(WS)
    args["bp"] = np.asarray(inputs["bp"], np.float32)
    pidx = np.arange(P)
    gmat = (pidx[:, None] // GSZ == np.arange(GPP)[None, :]).astype(np.float32)
    args["gmat"] = np.ascontiguousarray(gmat / float(GSZ))
    args["gtmat"] = np.ascontiguousarray(gmat.T)
    in_maps = []
    for core in range(8):
        bi, half = core // 2, core % 2
        sl = slice(half * NQ, (half + 1) * NQ)
        other = slice((1 - half) * NQ, (2 - half) * NQ)
        xp = np.concatenate([x[bi][:, sl], x[bi][:, other]], axis=1)
        in_maps.append({"xb": np.ascontiguousarray(xp.astype(F8NP)),
                        "xr": np.ascontiguousarray(x[bi][:, sl]), **args})

    from concourse.bass_utils import run_bass_kernel_spmd

    nc = build_program()
    trace = bool(int(os.environ.get("KERNEL_TRACE", "0")))
    res = run_bass_kernel_spmd(nc, in_maps, core_ids=list(range(8)),
                               trace=trace)
    kernel.last_results = res
    out = np.empty((B, C, L), np.float32)
    for core in range(8):
        bi, half = core // 2, core % 2
        out[bi][:, half * NQ:(half + 1) * NQ] = res.results[core]["out"]
    return out


# revision 33
# speedup vs baseline: 1.0028x; 1.0028x over previous
"""Trainium2 Bass kernel for nn_AttnBlock (GroupNorm + single-head attention
block over [b=4, c=512, l=4096] fp32, 8 NeuronCores).

Sharding: core = (batch, query-half). Each core gets one batch item with its
query half permuted to columns 0..2047 (GroupNorm/attention are invariant to
a consistent permutation of l), computes the full block for its 2048 query
positions, and the host reassembles the [4, 512, 4096] output.

Design (vs the v1 baseline, 346 -> ~222 us):
  - Weights pre-scaled by WS=16 and cast to fp8e4m3 on the HOST (kills 3 MB
    of prologue DMA + all on-chip weight casts); x shipped as fp8 in the
    DoubleRow layout and used RAW as the matmul operand - no separate
    GN-cast pass at all. All power-of-two compensations fold into existing
    scalar constants (exp scale, o_sb shrink, bias scales).
  - GroupNorm folded into the weights: per-input-channel scale m multiplies
    wk/wq/wv fp8 rows after stats; the GN bias term rides tiny N=1 PE
    matmuls (K's bias provably cancels in the softmax; V's folds into bp3;
    Q's lands in the PSUM-evacuation bias add).
  - GroupNorm stats split across engines and pipelined per channel block:
    DVE bn_stats for blocks 0/1/3a + ACT Square/Identity with accum_out for
    blocks 2/3b, x blocks arriving via three parallel DMA rings.
  - Attention software-pipelined: next step's S^T matmuls are emitted before
    this step's s/O matmuls so the PE never waits on exp() - the attention
    phase measures ~100% PE occupancy (<1 us total gaps).
  - s row-sums via an M=128 all-ones stationary => s broadcast to all
    partitions for free; 1/s on DVE directly; no PE broadcast matmul, no
    rank-1 bias-inject matmuls (bp3 pre-added into the residual tile).
  - Residual x streamed in f32 during the attention phase (HBM idle then);
    out = proj*rinv + (res+bp3) is a 2-op DVE chain.
  - PSUM: 3 (S^T) + 1 (s/bias) + 4 (O/proj) banks; PE warmup dummies paced
    by the stats stream pre-trigger the HAM clock un-throttle.
"""
import os
import sys
from contextlib import ExitStack

import numpy as np

sys.path.insert(0, "/opt/trn_rl_repo")

import concourse.bass as bass
import concourse.tile as tile
from concourse import bacc, mybir

F32 = mybir.dt.float32
BF16 = mybir.dt.bfloat16
F8 = mybir.dt.float8e4

B, C, L = 4, 512, 4096
NQ = L // 2          # queries per core
P = 128
CO = C // P          # 4 channel blocks
NJT = L // P         # 32 j-tiles
NIB = NQ // 512      # 4 i-blocks
NLC = L // 512       # 8 l-chunks
NG = 32              # groups
GSZ = C // NG        # 16 channels per group
GPP = P // GSZ       # 8 groups per 128 partitions
EPS = 1e-6
SCALE = float(C) ** -0.5
WS = 16.0            # host-side weight scale (power of two)
ESCALE = SCALE / (WS * WS)   # exp() input scale
OSC = 1.0 / 256.0    # O_u -> fp8 shrink, with the 1/WS^2 proj
                     # compensation folded in: out = pj * (1/s) exactly
HC = L // 2          # half-columns per x chunk

AF = mybir.ActivationFunctionType
ALU = mybir.AluOpType
DR = mybir.MatmulPerfMode.DoubleRow


def build_program():
    nc = bacc.Bacc("TRN2")
    x_d = nc.declare_dram_parameter("xb", [C, L], F8, isOutput=False)
    xr_d = nc.declare_dram_parameter("xr", [C, NQ], F32, isOutput=False)
    wq_d = nc.declare_dram_parameter("wq8", [C, C], F8, isOutput=False)
    wk_d = nc.declare_dram_parameter("wk8", [C, C], F8, isOutput=False)
    wv_d = nc.declare_dram_parameter("wv8", [C, C], F8, isOutput=False)
    wp_d = nc.declare_dram_parameter("wp8", [C, C], F8, isOutput=False)
    gns_d = nc.declare_dram_parameter("gn_scale", [C], F32, isOutput=False)
    gnb_d = nc.declare_dram_parameter("gn_bias", [C], F32, isOutput=False)
    bqw_d = nc.declare_dram_parameter("bqw", [C], F32, isOutput=False)
    bvw_d = nc.declare_dram_parameter("bvw", [C], F32, isOutput=False)
    bp_d = nc.declare_dram_parameter("bp", [C], F32, isOutput=False)
    gm_d = nc.declare_dram_parameter("gmat", [P, GPP], F32, isOutput=False)
    gt_d = nc.declare_dram_parameter("gtmat", [GPP, P], F32, isOutput=False)
    out_d = nc.declare_dram_parameter("out", [C, NQ], F32, isOutput=True)

    with tile.TileContext(nc) as tc:
        attn_block(tc, x_d, xr_d, wq_d, wk_d, wv_d, wp_d, gns_d, gnb_d,
                   bqw_d, bvw_d, bp_d, gm_d, gt_d, out_d)
    nc.compile()
    return nc


def attn_block(tc, x_d, xr_d, wq_d, wk_d, wv_d, wp_d, gns_d, gnb_d,
               bqw_d, bvw_d, bp_d, gm_d, gt_d, out_d):
    nc = tc.nc
    x_v = x_d.ap().rearrange("(a b p) l -> p a b l", a=2, b=2, p=P)
    xr_v = xr_d.ap().rearrange("(o p) i -> p o i", p=P)
    out_v = out_d.ap().rearrange("(o p) i -> p o i", p=P)

    with ExitStack() as ctx:
        # ---- persistent pools (whole kernel) ----
        big = ctx.enter_context(tc.tile_pool(name="big", bufs=1))
        small = ctx.enter_context(tc.tile_pool(name="small", bufs=1))

        x8 = big.tile([P, 2, 2, L], F8, tag="x8")      # GN-normalized x, fp8
        k8 = big.tile([P, 2, 2, L], F8, tag="k8")
        q8 = big.tile([P, 2, 2, NQ], F8, tag="q8")
        vt8 = big.tile([P, NJT // 2, 2, C], F8, tag="vt8")
        wq8 = big.tile([P, 2, 2, C], F8, tag="wq8")
        wk8 = big.tile([P, 2, 2, C], F8, tag="wk8")
        wv8 = big.tile([P, 2, 2, C], F8, tag="wv8")
        wp8 = big.tile([P, 2, 2, C], F8, tag="wp8")

        gns = small.tile([P, CO], F32, tag="gns")
        gnb = small.tile([P, CO], F32, tag="gnb")
        bqw = small.tile([P, CO], F32, tag="bqw")
        bp_s = small.tile([P, CO], F32, tag="bps")
        bp3 = small.tile([P, CO], F32, tag="bp3")
        bvw = small.tile([P, CO], F32, tag="bvw")
        bv2_8 = small.tile([P, 2, 2], F8, tag="bv28")
        a8 = small.tile([P, CO], F8, tag="a8")
        bq2 = small.tile([P, CO], F32, tag="bq2")
        m44 = small.tile([P, CO], F32, tag="m44")
        a44 = small.tile([P, CO], F32, tag="a44")
        gm_t = small.tile([P, GPP], F32, tag="gmt")
        gt_t = small.tile([GPP, P], F32, tag="gtt")
        ones_p = small.tile([P, 2, P], F8, tag="onesp")
        nc.vector.memset(ones_p, 1.0)
        nshift = small.tile([P, 1], F32, tag="nshift")
        nc.vector.memset(nshift, -3.0)
        eps_t = small.tile([GPP, 1], F32, tag="eps")
        nc.vector.memset(eps_t, EPS)
        warm8 = small.tile([P, 512], F8, tag="warm8")
        nc.vector.memset(warm8, 1.0)

        # static PSUM layout: 3 + 1 + 4 = 8 banks, shared across phases
        ps_st = ctx.enter_context(
            tc.tile_pool(name="psst", bufs=3, space="PSUM"))
        ps_s = ctx.enter_context(
            tc.tile_pool(name="pss", bufs=1, space="PSUM"))
        ps_o = ctx.enter_context(
            tc.tile_pool(name="pso", bufs=4, space="PSUM"))

        # small/weight DMAs on the gpsimd ring (x uses sync+scalar rings)
        for v_d, v_t in ((gns_d, gns), (gnb_d, gnb), (bqw_d, bqw),
                         (bp_d, bp_s), (bvw_d, bvw)):
            nc.gpsimd.dma_start(out=v_t[:], in_=v_d.ap().rearrange(
                "(o p) -> p o", p=P))

        nc.gpsimd.dma_start(out=gm_t[:], in_=gm_d.ap())
        nc.gpsimd.dma_start(out=gt_t[:], in_=gt_d.ap())

        # ====== prologue: raw-fp8 x, stats on DVE+ACT, GN folded into W ======
        with ExitStack() as pctx:
            pro = pctx.enter_context(tc.tile_pool(name="pro", bufs=1))
            tiny_ps = ps_st
            qkv_ps = ps_o

            bnst = pro.tile([P, CO, 8, 6], F32, tag="bnst")
            mv = pro.tile([P, CO, 2], F32, tag="mv")
            st2 = pro.tile([P, CO, 2], F32, tag="st2")
            sc2 = pro.tile([P, CO, 1], F32, tag="sc2")
            grp = pro.tile([GPP, CO, 6], F32, tag="grp")
            asum2 = pro.tile([P, 2, 2], F32, tag="asum2")
            asum3 = pro.tile([P, 2, 1], F32, tag="asum3")
            red2 = pro.tile([P, 2], F32, tag="red2")
            scr2 = pro.tile([P, 2, HC], F32, tag="scr2")

            # x blocks land via 4 parallel DMA rings in DoubleRow layout;
            # raw fp8 x is used directly as the matmul operand (GroupNorm
            # scale rides the weights, bias via tiny matmuls).
            nc.sync.dma_start(out=x8[:, 0, 0, :], in_=x_v[:, 0, 0, :])
            nc.scalar.dma_start(out=x8[:, 1, 0, :], in_=x_v[:, 1, 0, :])
            nc.gpsimd.dma_start(out=x8[:, 0, 1, :], in_=x_v[:, 0, 1, :])
            nc.scalar.dma_start(out=x8[:, 1, 1, :], in_=x_v[:, 1, 1, :])

            for w_d, w_t in ((wk_d, wk8), (wq_d, wq8), (wv_d, wv8),
                             (wp_d, wp8)):
                nc.gpsimd.dma_start(out=w_t[:], in_=w_d.ap().rearrange(
                    "(a b p) c -> p a b c", a=2, b=2, p=P))

            # block 2 stats on ACT, emitted first so they lead its queue:
            # two 2048-wide passes amortize the accumulator-read overhead
            for half in range(2):
                cols = slice(half * HC, (half + 1) * HC)
                nc.scalar.activation(out=scr2[:, half, :],
                                     in_=x8[:, 1, 0, cols], func=AF.Square,
                                     accum_out=asum2[:, 1, half:half + 1])
                nc.scalar.activation(out=scr2[:, half, :],
                                     in_=x8[:, 1, 0, cols], func=AF.Identity,
                                     accum_out=asum2[:, 0, half:half + 1])

            def block_chunk(o, h):
                return x8[:, o // 2, o % 2, h * 512:(h + 1) * 512]

            def combine(o):
                """group mean/rstd -> m44/a44/a8 col o (no folds here:
                keeping ACT's queue to bare sqrt ops lets the four chains
                pipeline instead of serializing behind weight folds)."""
                g_ps = tiny_ps.tile([GPP, 2], F32, tag="mm")
                nc.tensor.matmul(g_ps, lhsT=gm_t, rhs=st2[:, o, :],
                                 start=True, stop=True)
                nc.vector.tensor_copy(grp[:, o, 0:1], g_ps[:, 0:1])
                nc.vector.tensor_mul(grp[:, o, 2:3], grp[:, o, 0:1],
                                     grp[:, o, 0:1])
                nc.vector.tensor_sub(grp[:, o, 2:3], g_ps[:, 1:2],
                                     grp[:, o, 2:3])
                nc.scalar.activation(out=grp[:, o, 3:4], in_=grp[:, o, 2:3],
                                     func=AF.Sqrt, bias=eps_t)
                nc.vector.reciprocal_approx_accurate(
                    grp[:, o, 1:2], grp[:, o, 3:4], grp[:, o, 4:5])
                bc_ps = tiny_ps.tile([P, 2], F32, tag="mm")
                nc.tensor.matmul(bc_ps, lhsT=gt_t, rhs=grp[:, o, 0:2],
                                 start=True, stop=True)
                mcol = m44[:, o:o + 1]
                acol = a44[:, o:o + 1]
                nc.vector.tensor_mul(mcol, bc_ps[:, 1:2], gns[:, o:o + 1])
                nc.vector.tensor_mul(acol, bc_ps[:, 0:1], mcol)
                nc.vector.tensor_sub(acol, gnb[:, o:o + 1], acol)
                nc.vector.tensor_scalar_mul(a8[:, o:o + 1], acol, 64.0)

            def fold_and_bias(o):
                mcol = m44[:, o:o + 1]
                nc.scalar.activation(out=wk8[:, o // 2, o % 2, :],
                                     in_=wk8[:, o // 2, o % 2, :],
                                     func=AF.Copy, scale=mcol)
                for oc in range(CO):
                    nc.tensor.matmul(
                        bias_ps[:, oc:oc + 1],
                        lhsT=wq8[:, o // 2, o % 2, oc * P:(oc + 1) * P],
                        rhs=a8[:, o:o + 1],
                        start=(o == 0), stop=(o == CO - 1))
                for oc in range(CO):
                    nc.tensor.matmul(
                        bias_ps[:, 4 + oc:5 + oc],
                        lhsT=wv8[:, o // 2, o % 2, oc * P:(oc + 1) * P],
                        rhs=a8[:, o:o + 1],
                        start=(o == 0), stop=(o == CO - 1))

            bias_ps = ps_s.tile([P, 8], F32, tag="srow")
            ci = 0
            for o in (0, 1):  # DVE-owned blocks
                for hh in range(2):
                    for h in range(4):
                        nc.vector.bn_stats(
                            out=bnst[:, o, hh * 4 + h, :],
                            in_=block_chunk(o, hh * 4 + h))
                    # HAM warmup: dummy matmuls paced by the stats stream
                    nc.vector.tensor_copy(warm8[:, ci * 4:ci * 4 + 4],
                                          bnst[:, o, hh * 4 + 3, 0:4])
                    for _ in range(2 + 2 * o):
                        wm_ps = tiny_ps.tile([P, 512], F32, tag="mm")
                        nc.tensor.matmul(wm_ps, lhsT=warm8[:, 0:P],
                                         rhs=warm8[:], start=True, stop=True)
                    ci += 1
                nc.vector.bn_aggr(out=mv[:, o, :], in_=bnst[:, o, :, :])
                nc.vector.tensor_copy(st2[:, o, 0:1], mv[:, o, 0:1])
                nc.vector.tensor_mul(sc2[:, o, :], mv[:, o, 0:1],
                                     mv[:, o, 0:1])
                nc.vector.tensor_add(st2[:, o, 1:2], sc2[:, o, :],
                                     mv[:, o, 1:2])

            # block 2 merge (ACT accumulators -> mean / E[x^2])
            nc.vector.tensor_reduce(out=red2, in_=asum2,
                                    axis=mybir.AxisListType.X,
                                    op=ALU.add)
            nc.vector.tensor_scalar_mul(st2[:, 2, :], red2, 1.0 / L)

            # block 3 split: chunks 0-4 on DVE, cols 2560: on ACT
            nc.scalar.activation(out=scr2[:, 0, 0:1536],
                                 in_=x8[:, 1, 1, 2560:4096], func=AF.Square,
                                 accum_out=asum3[:, 1, :])
            nc.scalar.activation(out=scr2[:, 0, 0:1536],
                                 in_=x8[:, 1, 1, 2560:4096], func=AF.Identity,
                                 accum_out=asum3[:, 0, :])
            for h in range(5):
                nc.vector.bn_stats(out=bnst[:, 3, h, :],
                                   in_=block_chunk(3, h))
                if h in (1, 4):
                    nc.vector.tensor_copy(warm8[:, 32 + h * 4:36 + h * 4],
                                          bnst[:, 3, h, 0:4])
                    for _ in range(3):
                        wm_ps = tiny_ps.tile([P, 512], F32, tag="mm")
                        nc.tensor.matmul(wm_ps, lhsT=warm8[:, 0:P],
                                         rhs=warm8[:], start=True,
                                         stop=True)
            nc.vector.bn_aggr(out=mv[:, 3, :], in_=bnst[:, 3, 0:5, :])
            # st2 = (5/8)*dve_stats + act_sums/L
            nc.vector.tensor_mul(sc2[:, 3, :], mv[:, 3, 0:1], mv[:, 3, 0:1])
            nc.vector.tensor_add(sc2[:, 3, :], sc2[:, 3, :], mv[:, 3, 1:2])
            nc.vector.tensor_scalar_mul(red2, asum3[:, :, 0], 1.0 / L)
            nc.vector.tensor_scalar(out=st2[:, 3, 0:1], in0=mv[:, 3, 0:1],
                                    scalar1=5.0 / 8.0, scalar2=red2[:, 0:1],
                                    op0=ALU.mult, op1=ALU.add)
            nc.vector.tensor_scalar(out=st2[:, 3, 1:2], in0=sc2[:, 3, :],
                                    scalar1=5.0 / 8.0, scalar2=red2[:, 1:2],
                                    op0=ALU.mult, op1=ALU.add)

            # all stats emitted: now the four combine chains pipeline with
            # nothing else in the ACT/DVE queues, then folds + bias matmuls
            for o in range(CO):
                combine(o)
            for o in range(CO):
                fold_and_bias(o)

            # ---- finish bias path: bq2 / bv2_8 from accumulated matmuls ----
            for oc in range(CO):
                nc.vector.tensor_scalar(out=bq2[:, oc:oc + 1],
                                        in0=bias_ps[:, oc:oc + 1],
                                        scalar1=1.0 / 64.0,
                                        scalar2=bqw[:, oc:oc + 1],
                                        op0=ALU.mult, op1=ALU.add)
                nc.vector.tensor_scalar(
                    out=bv2_8[:, oc // 2, oc % 2:oc % 2 + 1],
                    in0=bias_ps[:, 4 + oc:5 + oc],
                    scalar1=1.0 / 64.0, scalar2=bvw[:, oc:oc + 1],
                    op0=ALU.mult, op1=ALU.add)
            # fold GN scale into wq/wv now that the bias matmuls read them
            for i, o in enumerate(range(CO)):
                eng = nc.vector if i % 2 == 0 else None
                mcol = m44[:, o:o + 1]
                if eng is None:
                    nc.scalar.activation(out=wq8[:, o // 2, o % 2, :],
                                         in_=wq8[:, o // 2, o % 2, :],
                                         func=AF.Copy, scale=mcol)
                    nc.scalar.activation(out=wv8[:, o // 2, o % 2, :],
                                         in_=wv8[:, o // 2, o % 2, :],
                                         func=AF.Copy, scale=mcol)
                else:
                    eng.tensor_scalar_mul(wq8[:, o // 2, o % 2, :],
                                          wq8[:, o // 2, o % 2, :], mcol)
                    eng.tensor_scalar_mul(wv8[:, o // 2, o % 2, :],
                                          wv8[:, o // 2, o % 2, :], mcol)

            # ---- bp3 = bp + wp @ (wv a + bv) : post-normalize bias ----
            for oc in range(CO):
                b_ps = tiny_ps.tile([P, 1], F32, tag="mm")
                for o in range(CO):
                    nc.tensor.matmul(b_ps,
                                     lhsT=wp8[:, o // 2, o % 2,
                                              oc * P:(oc + 1) * P],
                                     rhs=bv2_8[:, o // 2, o % 2:o % 2 + 1],
                                     start=(o == 0), stop=(o == CO - 1))
                nc.vector.tensor_scalar(out=bp3[:, oc:oc + 1], in0=b_ps,
                                        scalar1=1.0 / (WS * WS),
                                        scalar2=bp_s[:, oc:oc + 1],
                                        op0=ALU.mult, op1=ALU.add)

            # ---- Q / K / V^T from resident raw-fp8 x ----
            ev = 0
            for lc in range(NLC):
                l0 = lc * 512
                for oc in range(CO):
                    kp = qkv_ps.tile([P, 512], F32, tag="acc")
                    for pr in range(2):
                        nc.tensor.matmul(
                            kp, lhsT=wk8[:, pr, :, oc * P:(oc + 1) * P],
                            rhs=x8[:, pr, :, l0:l0 + 512],
                            start=(pr == 0), stop=(pr == 1), perf_mode=DR)
                    dst = k8[:, oc // 2, oc % 2, l0:l0 + 512]
                    if ev % 2 == 0:
                        nc.scalar.activation(out=dst, in_=kp, func=AF.Copy)
                    else:
                        nc.vector.tensor_copy(dst, kp)
                    ev += 1
                for jt in range(4):
                    j0 = l0 + jt * P
                    jtg = lc * 4 + jt
                    vp = qkv_ps.tile([P, C], F32, tag="acc")
                    for pr in range(2):
                        nc.tensor.matmul(
                            vp, lhsT=x8[:, pr, :, j0:j0 + P],
                            rhs=wv8[:, pr, :, :],
                            start=(pr == 0), stop=(pr == 1), perf_mode=DR)
                    dst = vt8[:, jtg // 2, jtg % 2, :]
                    if ev % 2 == 0:
                        nc.scalar.activation(out=dst, in_=vp, func=AF.Copy)
                    else:
                        nc.vector.tensor_copy(dst, vp)
                    ev += 1
                if lc < NIB:
                    for oc in range(CO):
                        qp = qkv_ps.tile([P, 512], F32, tag="acc")
                        for pr in range(2):
                            nc.tensor.matmul(
                                qp,
                                lhsT=wq8[:, pr, :, oc * P:(oc + 1) * P],
                                rhs=x8[:, pr, :, l0:l0 + 512],
                                start=(pr == 0), stop=(pr == 1),
                                perf_mode=DR)
                        dst = q8[:, oc // 2, oc % 2, l0:l0 + 512]
                        if ev % 2 == 0:
                            nc.scalar.activation(out=dst, in_=qp,
                                                 func=AF.Identity,
                                                 bias=bq2[:, oc:oc + 1])
                        else:
                            nc.vector.tensor_scalar_add(dst, qp,
                                                        bq2[:, oc:oc + 1])
                        ev += 1

        # ================= attention + proj per i-block =================
        with ExitStack() as actx:
            p_pool = actx.enter_context(tc.tile_pool(name="ppool", bufs=4))
            res_pool = actx.enter_context(tc.tile_pool(name="resp", bufs=2))
            osb_pool = actx.enter_context(tc.tile_pool(name="osb", bufs=2))
            out_pool = actx.enter_context(tc.tile_pool(name="outp", bufs=4))
            rinv_pool = actx.enter_context(tc.tile_pool(name="rinvp", bufs=2))

            NT = NJT // 2
            steps = [(ib, t) for ib in range(NIB) for t in range(NT)]

            def emit_scores(ib, t):
                """S^T matmuls + exp for step (ib, t) -> p_f8 tile."""
                i0 = ib * 512
                p_f8 = p_pool.tile([P, 2, 512], F8, tag="pbf")
                for ko in range(2):
                    jt = 2 * t + ko
                    st_ps = ps_st.tile([P, 512], F32, tag="mm")
                    for pr in range(2):
                        nc.tensor.matmul(
                            st_ps,
                            lhsT=k8[:, pr, :, jt * P:(jt + 1) * P],
                            rhs=q8[:, pr, :, i0:i0 + 512],
                            start=(pr == 0), stop=(pr == 1), perf_mode=DR)
                    # exp(S/sqrt(c) - 3): shift keeps P in fp8e4 range,
                    # cancels between the s-normalization and bp3 path.
                    nc.scalar.activation(
                        out=p_f8[:, ko, :], in_=st_ps, func=AF.Exp,
                        bias=nshift, scale=ESCALE)
                return p_f8

            s_ps = None
            o_ps = None
            p_cur = emit_scores(0, 0)
            for idx, (ib, t) in enumerate(steps):
                i0 = ib * 512
                if t == 0:
                    s_ps = ps_s.tile([P, 512], F32, tag="srow")
                    o_ps = [ps_o.tile([P, 512], F32, tag="acc",
                                      name=f"oacc{cc}") for cc in range(CO)]
                    res = res_pool.tile([P, CO, 512], F32, tag="res")
                    nc.sync.dma_start(out=res[:],
                                      in_=xr_v[:, :, i0:i0 + 512])
                    for cc in range(CO):
                        nc.vector.tensor_scalar_add(res[:, cc, :],
                                                    res[:, cc, :],
                                                    bp3[:, cc:cc + 1])
                # prefetch next step's scores: keeps PE fed while this
                # step's exp() drains on ACT
                p_next = (emit_scores(*steps[idx + 1])
                          if idx + 1 < len(steps) else None)
                nc.tensor.matmul(s_ps, lhsT=ones_p, rhs=p_cur,
                                 start=(t == 0), stop=(t == NT - 1),
                                 perf_mode=DR)
                for cc in range(CO):
                    nc.tensor.matmul(
                        o_ps[cc], lhsT=vt8[:, t, :, cc * P:(cc + 1) * P],
                        rhs=p_cur, start=(t == 0), stop=(t == NT - 1),
                        perf_mode=DR)
                p_cur = p_next
                if t < NT - 1:
                    continue
                # ---- epilogue: 1/s, fp8 O, proj, residual ----
                rinv = rinv_pool.tile([P, 512], F32, tag="rinv")
                nc.vector.reciprocal_approx_fast(rinv, s_ps)
                o_sb = osb_pool.tile([P, 2, 2, 512], F8, tag="osb")
                for cc in range(CO):
                    dst = o_sb[:, cc // 2, cc % 2, :]
                    if cc % 2 == 0:
                        nc.scalar.activation(out=dst, in_=o_ps[cc],
                                             func=AF.Copy, scale=OSC)
                    else:
                        nc.vector.tensor_scalar_mul(dst, o_ps[cc], OSC)
                for oc in range(CO):
                    pj_ps = ps_o.tile([P, 512], F32, tag="acc",
                                      name=f"pj{oc}")
                    for pr in range(2):
                        nc.tensor.matmul(
                            pj_ps,
                            lhsT=wp8[:, pr, :, oc * P:(oc + 1) * P],
                            rhs=o_sb[:, pr, :, :],
                            start=(pr == 0), stop=(pr == 1), perf_mode=DR)
                    out_t = out_pool.tile([P, 512], F32, tag="outt")
                    nc.vector.tensor_mul(out_t, pj_ps, rinv)
                    nc.vector.tensor_add(out_t, out_t, res[:, oc, :])
                    nc.sync.dma_start(out=out_v[:, oc, i0:i0 + 512],
                                      in_=out_t)


def kernel(**inputs):
    import ml_dtypes

    F8NP = ml_dtypes.float8_e4m3fn
    BF16NP = ml_dtypes.bfloat16
    x = np.ascontiguousarray(np.asarray(inputs["x"], np.float32))
    args = {}
    for nm, w in (("wq8", inputs["wq"]), ("wk8", inputs["wk"]),
                  ("wv8", inputs["wv"]), ("wp8", inputs["wp"])):
        wT = np.asarray(w, np.float32).T * WS
        args[nm] = np.ascontiguousarray(wT.astype(F8NP))
    args["gn_scale"] = np.asarray(inputs["gn_scale"], np.float32)
    args["gn_bias"] = np.asarray(inputs["gn_bias"], np.float32)
    args["bqw"] = np.asarray(inputs["bq"], np.float32) * np.float32(WS)
    args["bvw"] = np.asarray(inputs["bv"], np.float32) * np.float32(WS)
    args["bp"] = np.asarray(inputs["bp"], np.float32)
    pidx = np.arange(P)
    gmat = (pidx[:, None] // GSZ == np.arange(GPP)[None, :]).astype(np.float32)
    args["gmat"] = np.ascontiguousarray(gmat / float(GSZ))
    args["gtmat"] = np.ascontiguousarray(gmat.T)
    in_maps = []
    for core in range(8):
        bi, half = core // 2, core % 2
        sl = slice(half * NQ, (half + 1) * NQ)
        other = slice((1 - half) * NQ, (2 - half) * NQ)
        xp = np.concatenate([x[bi][:, sl], x[bi][:, other]], axis=1)
        in_maps.append({"xb": np.ascontiguousarray(xp.astype(F8NP)),
                        "xr": np.ascontiguousarray(x[bi][:, sl]), **args})

    from concourse.bass_utils import run_bass_kernel_spmd

    nc = build_program()
    trace = bool(int(os.environ.get("KERNEL_TRACE", "0")))
    res = run_bass_kernel_spmd(nc, in_maps, core_ids=list(range(8)),
                               trace=trace)
    kernel.last_results = res
    out = np.empty((B, C, L), np.float32)
    for core in range(8):
        bi, half = core // 2, core % 2
        out[bi][:, half * NQ:(half + 1) * NQ] = res.results[core]["out"]
    return out
